# revision 3
# baseline (speedup 1.0000x reference)
"""Trainium2 Bass kernel for nn_ArgumentLogits (ragged argument logits head), v2.

Self-contained: hardcodes all shapes. Strategy:
 - batch-parallel local phase (8 cores x 32 batches), def-parallel global
   phase (each core owns 2500 of the 20000 definitions and computes logits
   for ALL arguments against its def slice). This cuts the replicated
   embedding-key read 8x vs pure batch parallelism.
 - local logits use q' = W_key^T q per argument (transform the 8 queries per
   batch instead of the ~512 context nodes), so the big ctx tensor streams
   through the PE exactly once and no [128, n_ctx] keys intermediate exists.
 - every core computes queries for all 256 batches (cheap: ~35k PE cycles);
   its own 32 batches are packed first in stin so the local phase uses
   compile-time AP offsets (SPMD-safe). Ragged arg compaction is done with
   a gpsimd ap_gather driven by a host-provided index tensor (data, not
   code, so the graph stays identical across cores).
 - per-batch context lengths use a tiered profile T[i] (max over cores of
   the i-th longest length), so padding is ~5% instead of ~25% while AP
   offsets stay compile-time constants shared by all cores.
 - embedding-key norms: ACT square+accumulate over a def-major copy of the
   core's gk slice; 1/(1e-7+sqrt()) is applied as a per-partition scale
   fused into the global PSUM eviction.
"""

import math
import time

import numpy as np
import ml_dtypes

import concourse.bass as bass
import concourse.mybir as mybir
import concourse.tile as tile
from concourse import bacc
from concourse.bass_utils import run_bass_kernel_spmd

BS = 256
MAX_ARGS = 8
CTX_DIM = 128
NODE_DIM = 128
HIDDEN = 512
STATE_DIM = 512
TAC_DIM = 128
TOTAL_CTX = 131072
N_CLASS = 30000
DEF_NUM = 20000
CTX_VAL_DIM = 256
DIM = CTX_DIM + 1 + NODE_DIM  # 257
N_CORES = 8
BPC = BS // N_CORES            # 32 batches per core
DPC = DEF_NUM // N_CORES       # 2500 defs per core
NDT = (DPC + 127) // 128       # 20 def tiles per core
DPC_PAD = NDT * 128            # 2560
NSLOT = BS * MAX_ARGS          # 2048 arg slots

BF16 = mybir.dt.bfloat16
F32 = mybir.dt.float32
I16 = mybir.dt.int16
NP_BF16 = ml_dtypes.bfloat16

FLOAT_KEYS = ("ctx_vals", "state_emb", "tactic_emb", "emb_table", "W_key",
              "b_key", "W_st", "b_st", "W_q", "b_q")


# ---------------------------------------------------------------- host plumbing

def _build_indices(ctx_ids, arg_cnt):
    """Mirror of the reference's host-side ragged index reconstruction."""
    ctx_ids = np.asarray(ctx_ids)
    arg_cnt = np.asarray(arg_cnt)
    arguments_i = np.repeat(np.arange(BS), arg_cnt)
    total_args = arguments_i.shape[0]
    ctx_lens = np.bincount(ctx_ids, minlength=BS)
    ctx_starts = np.concatenate([[0], np.cumsum(ctx_lens)[:-1]])
    arg_ctx_lens = ctx_lens[arguments_i]
    rows = np.repeat(np.arange(total_args), arg_ctx_lens)
    return arguments_i, total_args, ctx_lens, ctx_starts, arg_ctx_lens, rows


def _plan(ctx_ids, arg_cnt):
    """Batch->core assignment plus the shared tier profile T[i]."""
    arg_cnt = np.asarray(arg_cnt)
    ctx_lens = np.bincount(np.asarray(ctx_ids), minlength=BS)
    # snake-deal batches (sorted by length desc) into 8 cores so each core's
    # sorted length profile is nearly identical -> tight common tier profile
    order = np.argsort(-ctx_lens, kind="stable")
    core_batches = [[] for _ in range(N_CORES)]
    for r in range(BPC):
        row = order[r * N_CORES:(r + 1) * N_CORES]
        if r % 2:
            row = row[::-1]
        for c in range(N_CORES):
            core_batches[c].append(int(row[c]))
    # tier profile: T[i] = max over cores of i-th longest, padded to 4
    T = np.zeros(BPC, np.int64)
    for c in range(N_CORES):
        lens = ctx_lens[core_batches[c]]          # already desc-ish
        lens = np.sort(lens)[::-1]
        core_batches[c] = sorted(core_batches[c],
                                 key=lambda b: -ctx_lens[b])
        T = np.maximum(T, lens)
    T = ((T + 3) // 4) * 4
    T = np.maximum(T, 4)
    total_args = int(arg_cnt.sum())
    a_pad = max(512, ((total_args + 127) // 128) * 128)
    assert a_pad <= 2048
    return dict(core_batches=core_batches, T=[int(x) for x in T],
                a_pad=a_pad, ctx_lens=ctx_lens, total_args=total_args)


# ---------------------------------------------------------------- device graph

_GRAPH_CACHE = {}


def build_graph(T, a_pad, replicas=1, shared_out=False, loads_in_loop=False):
    T = tuple(T)
    key = (T, a_pad, replicas, shared_out, loads_in_loop)
    if key in _GRAPH_CACHE:
        return _GRAPH_CACHE[key]

    A = a_pad
    OFF = [0]
    for t in T:
        OFF.append(OFF[-1] + t)
    CTOT = OFF[-1]                       # packed ctx cols per core
    GOFF = [OFF[4 * g] for g in range(9)]  # group boundaries (8 groups of 4)
    # local-stage layout: group g occupies cols [LOFF[g], LOFF[g]+T[4g])
    # (all 4 batches of the group share the column range, one per 32-partition
    # band, mirroring the PSUM tile)
    LOFF = [0]
    for g in range(8):
        LOFF.append(LOFF[-1] + T[4 * g])
    LTOT = LOFF[-1]
    NA_CH = A // 512                     # arg chunks of 512 in global phase

    nc = bacc.Bacc("TRN2", target_bir_lowering=False, debug=False)

    # ---- inputs (identical shapes on every core; raggedness is in the data)
    ctxT = nc.dram_tensor("ctxT", [2, 128, CTOT], BF16, kind="ExternalInput")
    stinT = nc.dram_tensor("stinT", [128, 5, BS], BF16, kind="ExternalInput")
    wst = nc.dram_tensor("wst", [128, 5, HIDDEN], BF16, kind="ExternalInput")
    wq = nc.dram_tensor("wq", [128, 4, MAX_ARGS * DIM], BF16, kind="ExternalInput")
    wqn = nc.dram_tensor("wqn", [128, 4, MAX_ARGS], BF16, kind="ExternalInput")
    wkeyT = nc.dram_tensor("wkeyT", [128, 2, 128], BF16, kind="ExternalInput")
    bkeyC = nc.dram_tensor("bkeyC", [128, 1], BF16, kind="ExternalInput")
    b_stT = nc.dram_tensor("b_stT", [128, 4], F32, kind="ExternalInput")
    bq_locT = nc.dram_tensor("bq_locT", [128, MAX_ARGS], F32, kind="ExternalInput")
    bq_gloT = nc.dram_tensor("bq_gloT", [128, MAX_ARGS], F32, kind="ExternalInput")
    b_noneC = nc.dram_tensor("b_noneC", [MAX_ARGS, 1], F32, kind="ExternalInput")
    gkT = nc.dram_tensor("gkT", [128, DPC_PAD], BF16, kind="ExternalInput")
    gk_dm = nc.dram_tensor("gk_dm", [128, NDT, 128], BF16, kind="ExternalInput")
    gidx = nc.dram_tensor("gidx", [128, A // 16], I16, kind="ExternalInput")
    tick = nc.dram_tensor("tick", [128, 8], F32, kind="ExternalInput")

    R = 1 if shared_out else replicas
    if R == 1:
        out_glob = nc.dram_tensor("out_glob", [NDT, 128, A], BF16,
                                  kind="ExternalOutput")
        out_local = nc.dram_tensor("out_local", [128, LTOT], BF16,
                                   kind="ExternalOutput")
        out_none = nc.dram_tensor("out_none", [MAX_ARGS, BS], F32,
                                  kind="ExternalOutput")
        tock = nc.dram_tensor("tock", [128, 8], F32, kind="ExternalOutput")
    else:
        out_glob_r = nc.dram_tensor("out_glob", [R, NDT, 128, A], BF16,
                                    kind="ExternalOutput")
        out_local_r = nc.dram_tensor("out_local", [R, 128, LTOT], BF16,
                                     kind="ExternalOutput")
        out_none_r = nc.dram_tensor("out_none", [R, MAX_ARGS, BS], F32,
                                    kind="ExternalOutput")
        tock_r = nc.dram_tensor("tock", [R, 128, 8], F32, kind="ExternalOutput")

    with tile.TileContext(nc) as tc:
        with (
            tc.tile_pool(name="persist", bufs=1) as persist,
            tc.tile_pool(name="gstage", bufs=3) as gstage,
            tc.tile_pool(name="pa", bufs=3, space="PSUM") as pa,
            tc.tile_pool(name="pb", bufs=2, space="PSUM") as pb,
        ):
            # ---- resident inputs
            stin_sb = persist.tile([128, 5, BS], BF16, tag="stin")
            wst_sb = persist.tile([128, 5, HIDDEN], BF16, tag="wst")
            wq_sb = persist.tile([128, 4, MAX_ARGS * DIM], BF16, tag="wq")
            wqn_sb = persist.tile([128, 4, MAX_ARGS], BF16, tag="wqn")
            wkeyT_sb = persist.tile([128, 2, 128], BF16, tag="wkeyT")
            bkey_sb = persist.tile([128, 1], BF16, tag="bkey")
            bst_sb = persist.tile([128, 4], F32, tag="bst")
            bloc_sb = persist.tile([128, MAX_ARGS], F32, tag="bloc")
            bglo_sb = persist.tile([128, MAX_ARGS], F32, tag="bglo")
            bnone_sb = persist.tile([MAX_ARGS, 1], F32, tag="bnone")
            gidx_sb = persist.tile([128, A // 16], I16, tag="gidx")
            gkT_sb = persist.tile([128, DPC_PAD], BF16, tag="gkT")
            gkdm_sb = persist.tile([128, NDT, 128], BF16, tag="gkdm")
            ctx_sb = persist.tile([128, 2, CTOT], BF16, tag="ctx")

            def emit_loads():
                nc.sync.dma_start(stin_sb[:], stinT[:])
                nc.sync.dma_start(wst_sb[:], wst[:])
                nc.sync.dma_start(wq_sb[:], wq[:])
                nc.sync.dma_start(wqn_sb[:], wqn[:])
                nc.sync.dma_start(wkeyT_sb[:], wkeyT[:])
                nc.sync.dma_start(bkey_sb[:], bkeyC[:])
                nc.sync.dma_start(bst_sb[:], b_stT[:])
                nc.sync.dma_start(bloc_sb[:], bq_locT[:])
                nc.sync.dma_start(bglo_sb[:], bq_gloT[:])
                nc.sync.dma_start(bnone_sb[:], b_noneC[:])
                nc.sync.dma_start(gidx_sb[:], gidx[:])
                nc.scalar.dma_start(gkT_sb[:], gkT[:])
                nc.scalar.dma_start(gkdm_sb[:], gk_dm[:])
                q4 = [GOFF[0], GOFF[2], GOFF[4], GOFF[6], GOFF[8]]
                for ci in range(4):
                    nc.sync.dma_start(ctx_sb[:, :, q4[ci]:q4[ci + 1]],
                                      ctxT[:, :, q4[ci]:q4[ci + 1]]
                                      .rearrange("h p c -> p h c"))

            if not loads_in_loop:
                emit_loads()

            # tick->tock passthrough: defeats CSE when chaining bench calls
            tick_sb = persist.tile([128, 8], F32, tag="tick")
            nc.gpsimd.dma_start(tick_sb[:], tick[:])
            nc.vector.tensor_scalar_add(tick_sb[:], tick_sb[:], 1.0)
            tock0 = tock[:] if R == 1 else tock_r[:].rearrange(
                "r p n -> (r p) n")[:128]
            nc.gpsimd.dma_start(tock0, tick_sb[:])

            # persistent compute tiles (shared across replicas)
            st_sb = persist.tile([128, 4, BS], BF16, tag="st")
            gq_f32 = persist.tile([128, NSLOT, 1], F32, tag="gqf")
            gq_sel = persist.tile([128, A, 1], F32, tag="gqsel")
            gq_bf = persist.tile([128, A], BF16, tag="gqbf")
            qT_sb = persist.tile([128, BPC * MAX_ARGS], BF16, tag="qT")
            qp_sb = persist.tile([128, 2, BPC * MAX_ARGS], BF16, tag="qp")
            qbrow = persist.tile([1, BPC * MAX_ARGS], F32, tag="qbrow")
            qb_part = persist.tile([128, 8], F32, tag="qbpart")
            none_sb = persist.tile([MAX_ARGS, BS], F32, tag="none")
            sumsq = persist.tile([128, NDT], F32, tag="sumsq")
            inv = persist.tile([128, NDT], F32, tag="inv")
            sqjunk = persist.tile([128, 128], BF16, tag="sqjunk")
            lstage = persist.tile([128, LTOT], BF16, tag="lstage")

            for _rep in range(replicas):
                if loads_in_loop:
                    emit_loads()
                if replicas > 1 and not shared_out:
                    out_glob = out_glob_r[_rep]
                    out_local = out_local_r[_rep]
                    out_none = out_none_r[_rep]

                # ---- phase 1: st = relu(stin @ W_st + b_st)  [128,4,BS]
                for m in range(4):
                    ps = pb.tile([128, BS], F32, tag="pb")
                    for k in range(5):
                        nc.tensor.matmul(ps[:], wst_sb[:, k, m * 128:(m + 1) * 128],
                                         stin_sb[:, k, :], start=(k == 0),
                                         stop=(k == 4))
                    nc.scalar.activation(st_sb[:, m, :], ps[:],
                                         mybir.ActivationFunctionType.Relu,
                                         bias=bst_sb[:, m:m + 1])

                # ---- phase 2: global queries for ALL slots (dims-major, f32)
                gqf_v = gq_f32[:, :, 0].rearrange("p (b j) -> p b j", j=MAX_ARGS)
                for j in range(MAX_ARGS):
                    c0 = j * DIM + CTX_DIM + 1
                    ps = pb.tile([128, BS], F32, tag="pb")
                    for k in range(4):
                        nc.tensor.matmul(ps[:], wq_sb[:, k, c0:c0 + NODE_DIM],
                                         st_sb[:, k, :], start=(k == 0),
                                         stop=(k == 3))
                    if j % 2 == 0:
                        nc.scalar.activation(gqf_v[:, :, j], ps[:],
                                             mybir.ActivationFunctionType.Identity,
                                             bias=bglo_sb[:, j:j + 1])
                    else:
                        nc.vector.tensor_scalar_add(gqf_v[:, :, j], ps[:],
                                                    bglo_sb[:, j:j + 1])

                # none logits for all slots
                psn = pb.tile([MAX_ARGS, BS], F32, tag="pb")
                for k in range(4):
                    nc.tensor.matmul(psn[:], wqn_sb[:, k, :], st_sb[:, k, :],
                                     start=(k == 0), stop=(k == 3))
                nc.scalar.activation(none_sb[:], psn[:],
                                     mybir.ActivationFunctionType.Identity,
                                     bias=bnone_sb[:, 0:1])
                nc.gpsimd.dma_start(out_none[:], none_sb[:])

                # ---- phase 3: local queries (own 32 batches = stin cols 0:32)
                qT_v = qT_sb[:].rearrange("p (b j) -> p b j", j=MAX_ARGS)
                for j in range(MAX_ARGS):
                    c0 = j * DIM
                    ps = pb.tile([128, BPC], F32, tag="pb")
                    for k in range(4):
                        nc.tensor.matmul(ps[:], wq_sb[:, k, c0:c0 + CTX_DIM],
                                         st_sb[:, k, :BPC], start=(k == 0),
                                         stop=(k == 3))
                    nc.scalar.activation(qT_v[:, :, j], ps[:],
                                         mybir.ActivationFunctionType.Identity,
                                         bias=bloc_sb[:, j:j + 1])

                # q' = W_key^T q  (per-arg 256-dim local key queries)
                for h in range(2):
                    psq = pb.tile([128, BPC * MAX_ARGS], F32, tag="pb")
                    nc.tensor.matmul(psq[:], wkeyT_sb[:, h, :], qT_sb[:],
                                     start=True, stop=True)
                    nc.vector.tensor_copy(qp_sb[:, h, :], psq[:])
                # qb = b_key . q  (scalar per local arg slot), computed with
                # the moving operand pre-shuffled to (k, j, g) order so the
                # repartition below is a plain strided DMA
                psb = pb.tile([1, BPC * MAX_ARGS], F32, tag="pb")
                qT_shuf = bass.AP(tensor=qT_sb[:].tensor,
                                  offset=qT_sb[:].offset,
                                  ap=[list(qT_sb[:].ap[0]),
                                      [8, 4], [1, 8], [32, 8]])
                nc.tensor.matmul(psb[:], bkey_sb[:], qT_shuf,
                                 start=True, stop=True)
                nc.scalar.activation(qbrow[:], psb[:],
                                     mybir.ActivationFunctionType.Identity)
                # repartition qb: shuffled col 64k+8j+g -> partition 32k+j, col g
                for k4 in range(4):
                    nc.sync.dma_start(qb_part[32 * k4:32 * k4 + 8, :],
                                      qbrow[0:1, 64 * k4:64 * k4 + 64])

                # ---- phase 4: compact global queries to ref arg order
                nc.gpsimd.ap_gather(gq_sel[:], gq_f32[:], gidx_sb[:],
                                    channels=128, num_elems=NSLOT, d=1,
                                    num_idxs=A)
                nc.vector.tensor_copy(gq_bf[:], gq_sel[:, :, 0])

                # ---- phase 5: embedding-key norms from the def-major copy
                for t in range(NDT):
                    nc.scalar.activation(sqjunk[:], gkdm_sb[:, t, :],
                                         mybir.ActivationFunctionType.Square,
                                         accum_out=sumsq[:, t:t + 1])
                nc.scalar.activation(inv[:], sumsq[:],
                                     mybir.ActivationFunctionType.Sqrt)
                nc.vector.tensor_scalar_add(inv[:], inv[:], 1e-7)
                nc.vector.reciprocal(inv[:], inv[:])

                # ---- phase 6+7 interleaved: global def tiles + local groups.
                # global tile t: [128 defs, A args]; local group g: 4 batches
                # at partition offsets 32k, full tier width, 256-dim contraction
                def emit_global(t):
                    for ac in range((A + 1023) // 1024):
                        a0 = ac * 1024
                        aw = min(1024, A - a0)
                        psg = pa.tile([128, 1024], F32, tag="pa")
                        for s0 in range(0, aw, 512):
                            nc.tensor.matmul(
                                psg[:, s0:s0 + min(512, aw - s0)],
                                gkT_sb[:, t * 128:(t + 1) * 128],
                                gq_bf[:, a0 + s0:a0 + s0 + min(512, aw - s0)],
                                start=True, stop=True)
                        stg = gstage.tile([128, 1024], BF16, tag="gst")
                        if t % 2 == 0:
                            nc.scalar.mul(stg[:, :aw], psg[:, :aw],
                                          inv[:, t:t + 1])
                        else:
                            nc.vector.tensor_scalar_mul(stg[:, :aw], psg[:, :aw],
                                                        inv[:, t:t + 1])
                        nc.gpsimd.dma_start(out_glob[t, :, a0:a0 + aw],
                                            stg[:, :aw])

                def emit_local(g):
                    g0 = LOFF[g]
                    gmax = T[4 * g]          # largest tier in this group
                    psl = pa.tile([128, 1024], F32, tag="pa")
                    for k in range(4):
                        b = 4 * g + k
                        bw = T[b]
                        boff = OFF[b]
                        for n0 in range(0, bw, 512):
                            nw = min(512, bw - n0)
                            for h in range(2):
                                nc.tensor.matmul(
                                    psl[32 * k:32 * k + 8, n0:n0 + nw],
                                    qp_sb[:, h, b * MAX_ARGS:(b + 1) * MAX_ARGS],
                                    ctx_sb[:, h, boff + n0:boff + n0 + nw],
                                    start=(h == 0), stop=(h == 1),
                                    tile_position=(0, 32 * k))
                    # evict with qb bias; junk partitions (j>=8 lanes) are
                    # written but never leave the chip
                    psl_v = psl[:, :gmax]
                    if g % 2 == 0:
                        nc.scalar.activation(lstage[:, g0:g0 + gmax], psl_v,
                                             mybir.ActivationFunctionType.Identity,
                                             bias=qb_part[:, g:g + 1])
                    else:
                        nc.vector.tensor_scalar_add(lstage[:, g0:g0 + gmax],
                                                    psl_v, qb_part[:, g:g + 1])

                gq_done = 0
                for t in range(NDT):
                    emit_global(t)
                    # interleave local groups between global tiles
                    if t % 2 == 1 and gq_done < 8:
                        emit_local(gq_done)
                        gq_done += 1
                while gq_done < 8:
                    emit_local(gq_done)
                    gq_done += 1

                # local out: 4 DMAs, one per within-group batch position k
                lst_v = lstage[:].rearrange("(k j) c -> k j c", k=4)
                outl_v = out_local[:].rearrange("(k j) c -> k j c", k=4)
                for k4 in range(4):
                    nc.gpsimd.dma_start(outl_v[k4, :MAX_ARGS, :],
                                        lst_v[k4, :MAX_ARGS, :])

    nc.compile()
    _GRAPH_CACHE[key] = nc
    return nc


# ---------------------------------------------------------------- input packing

def pack_inputs(plan, inputs):
    T = plan["T"]
    A = plan["a_pad"]
    ctx_lens = plan["ctx_lens"]
    ctx_starts = np.concatenate([[0], np.cumsum(ctx_lens)[:-1]])
    arg_cnt = np.asarray(inputs["arg_cnt"])
    OFF = np.concatenate([[0], np.cumsum(T)]).astype(np.int64)
    CTOT = int(OFF[-1])

    f = {k: np.asarray(inputs[k], np.float32) for k in FLOAT_KEYS}
    gc = np.asarray(inputs["global_context"])

    # shared (replicated) tensors
    W_st, b_st, W_q, b_q = f["W_st"], f["b_st"], f["W_q"], f["b_q"]
    wst_r = np.ascontiguousarray(
        W_st.reshape(5, 128, HIDDEN).transpose(1, 0, 2)).astype(NP_BF16)
    wq_r = np.ascontiguousarray(
        W_q.reshape(4, 128, MAX_ARGS * DIM).transpose(1, 0, 2)).astype(NP_BF16)
    none_cols = [j * DIM + CTX_DIM for j in range(MAX_ARGS)]
    wqn_r = np.ascontiguousarray(
        W_q[:, none_cols].reshape(4, 128, MAX_ARGS).transpose(1, 0, 2)
    ).astype(NP_BF16)
    # W_key^T: wkeyT[d, h, f] = W_key[128h+f, d]
    wkeyT_r = np.ascontiguousarray(
        f["W_key"].reshape(2, 128, CTX_DIM).transpose(2, 0, 1)).astype(NP_BF16)
    bkey_c = f["b_key"].reshape(128, 1).astype(NP_BF16)
    b_stT = np.ascontiguousarray(b_st.reshape(4, 128).T)
    bq_locT = np.stack([b_q[j * DIM:j * DIM + CTX_DIM] for j in range(MAX_ARGS)],
                       axis=1)
    bq_gloT = np.stack([b_q[j * DIM + CTX_DIM + 1:(j + 1) * DIM]
                        for j in range(MAX_ARGS)], axis=1)
    b_noneC = b_q[none_cols].reshape(MAX_ARGS, 1)

    gk_raw = f["emb_table"][gc]  # [20000, 128] host gather (data movement)
    ctx_vals = f["ctx_vals"]
    state_emb, tactic_emb = f["state_emb"], f["tactic_emb"]

    in_maps = []
    for c in range(N_CORES):
        bl = plan["core_batches"][c]
        rest = [b for b in range(BS) if b not in set(bl)]
        order = bl + rest                       # core's own batches first
        pos = {b: i for i, b in enumerate(order)}

        # ctx packed by tier profile
        big = np.zeros((CTOT, CTX_VAL_DIM), np.float32)
        for i, b in enumerate(bl):
            L = int(ctx_lens[b])
            s0 = int(ctx_starts[b])
            assert L <= T[i]
            big[OFF[i]:OFF[i] + L] = ctx_vals[s0:s0 + L]
        ctxT = np.ascontiguousarray(big.T).reshape(2, 128, CTOT).astype(NP_BF16)

        stin = np.concatenate([state_emb[order], tactic_emb[order]], axis=1)
        stinT = np.ascontiguousarray(
            stin.T.reshape(5, 128, BS).transpose(1, 0, 2)).astype(NP_BF16)

        # arg -> slot gather indices (reference arg order), wrapped per 16
        flat_idx = np.zeros(A, np.int16)
        a = 0
        for b in range(BS):
            for j in range(int(arg_cnt[b])):
                flat_idx[a] = pos[b] * MAX_ARGS + j
                a += 1
        gidx = np.zeros((128, A // 16), np.int16)
        for grp in range(8):
            for p in range(16):
                gidx[16 * grp + p, :] = flat_idx[p::16]

        # def slice, both orientations
        gk_pad = np.zeros((DPC_PAD, NODE_DIM), np.float32)
        gk_pad[:DPC] = gk_raw[c * DPC:(c + 1) * DPC]
        gkT_r = np.ascontiguousarray(gk_pad.T).astype(NP_BF16)
        gk_dm_r = np.ascontiguousarray(
            gk_pad.reshape(NDT, 128, NODE_DIM).transpose(1, 0, 2)
        ).astype(NP_BF16)

        in_maps.append(dict(
            ctxT=ctxT, stinT=stinT, wst=wst_r, wq=wq_r, wqn=wqn_r,
            wkeyT=wkeyT_r, bkeyC=bkey_c, b_stT=b_stT, bq_locT=bq_locT,
            bq_gloT=bq_gloT, b_noneC=b_noneC, gkT=gkT_r, gk_dm=gk_dm_r,
            gidx=gidx, tick=np.zeros((128, 8), np.float32),
        ))
    return in_maps


# ---------------------------------------------------------------- assembly

def assemble(plan, results, ctx_ids, arg_cnt):
    arg_cnt = np.asarray(arg_cnt)
    (arguments_i, total_args, ctx_lens, ctx_starts,
     arg_ctx_lens, rows) = _build_indices(ctx_ids, arg_cnt)
    T = plan["T"]
    A = plan["a_pad"]
    LOFF = np.concatenate([[0], np.cumsum([T[4 * g] for g in range(8)])]
                          ).astype(np.int64)

    # where is each global batch: (core, local idx)
    where = {}
    for c in range(N_CORES):
        for i, b in enumerate(plan["core_batches"][c]):
            where[b] = (c, i)

    loc_parts = []
    none_parts = []
    none0 = results[0]["out_none"]
    pos0 = {}
    bl0 = plan["core_batches"][0]
    rest0 = [b for b in range(BS) if b not in set(bl0)]
    for i, b in enumerate(bl0 + rest0):
        pos0[b] = i
    for b in range(BS):
        c, i = where[b]
        L = int(ctx_lens[b])
        ol = results[c]["out_local"]          # [128, LTOT]
        k, g = i % 4, i // 4
        for j in range(int(arg_cnt[b])):
            loc_parts.append(
                ol[32 * k + j, LOFF[g]:LOFF[g] + L].astype(np.float32))
            none_parts.append(none0[j, pos0[b]])

    local_flat = (np.concatenate(loc_parts) if loc_parts
                  else np.zeros((0,), np.float32))
    none_logits = np.asarray(none_parts, np.float32)

    # global: core c owns defs [c*DPC, (c+1)*DPC), args already in ref order
    glob = np.empty((total_args, DEF_NUM), np.float32)
    for c in range(N_CORES):
        og = results[c]["out_glob"].reshape(NDT * 128, A)
        glob[:, c * DPC:(c + 1) * DPC] = og[:DPC, :total_args].astype(np.float32).T

    values = np.concatenate([local_flat, none_logits, glob.reshape(-1)])
    indices = np.concatenate([
        rows.astype(np.int32),
        np.arange(total_args, dtype=np.int32),
        np.repeat(np.arange(total_args, dtype=np.int32), DEF_NUM)])
    return indices, values.astype(np.float32)


# ---------------------------------------------------------------- entry points

_LAST = {}


def kernel(**inputs):
    ctx_ids = np.asarray(inputs["ctx_ids"])
    arg_cnt = np.asarray(inputs["arg_cnt"])
    plan = _plan(ctx_ids, arg_cnt)
    nc = build_graph(plan["T"], plan["a_pad"])
    in_maps = pack_inputs(plan, inputs)
    res = run_bass_kernel_spmd(nc, in_maps, core_ids=list(range(N_CORES)))
    _LAST.update(nc=nc, in_maps=in_maps, plan=plan)
    return assemble(plan, res.results, ctx_ids, arg_cnt)


def _run_once_timer(nc, in_maps, reps=12):
    """Single-bind jitted runner with device-resident inputs; min wall secs."""
    import jax
    from jax.sharding import Mesh, PartitionSpec, NamedSharding
    from jax.experimental.shard_map import shard_map
    from concourse.bass2jax import (_bass_exec_p, install_neuronx_cc_hook,
                                    partition_id_tensor)

    install_neuronx_cc_hook()
    part_name = nc.partition_id_tensor.name if nc.partition_id_tensor else None
    in_names, out_names, out_avals, zero_outs = [], [], [], []
    for alloc in nc.m.functions[0].allocations:
        if not isinstance(alloc, mybir.MemoryLocationSet):
            continue
        name = alloc.memorylocations[0].name
        if alloc.kind == "ExternalInput":
            if name != part_name:
                in_names.append(name)
        elif alloc.kind == "ExternalOutput":
            out_names.append(name)
            shape = tuple(alloc.tensor_shape)
            dtype = mybir.dt.np(alloc.dtype)
            out_avals.append(jax.core.ShapedArray(shape, dtype))
            zero_outs.append(np.zeros(shape, dtype))
    n_params = len(in_names)
    bind_names = in_names + out_names + ([part_name] if part_name else [])

    def _body(*args):
        operands = list(args)
        if part_name:
            operands.append(partition_id_tensor())
        outs = _bass_exec_p.bind(
            *operands,
            out_avals=tuple(out_avals),
            in_names=tuple(bind_names),
            out_names=tuple(out_names),
            lowering_input_output_aliases=(),
            sim_require_finite=True,
            sim_require_nnan=True,
            nc=nc,
        )
        return tuple(outs)

    devices = jax.devices()[:N_CORES]
    mesh = Mesh(np.asarray(devices), ("core",))
    specs = (PartitionSpec("core"),) * (n_params + len(out_names))
    out_specs = (PartitionSpec("core"),) * len(out_names)
    shd = NamedSharding(mesh, PartitionSpec("core"))

    concat_in = [jax.device_put(
        np.concatenate([np.asarray(in_maps[c][k]) for c in range(N_CORES)],
                       axis=0), shd) for k in in_names]
    concat_zero = [jax.device_put(
        np.zeros((N_CORES * z.shape[0], *z.shape[1:]), z.dtype), shd)
        for z in zero_outs]
    jax.block_until_ready(concat_in)
    jax.block_until_ready(concat_zero)

    fn = jax.jit(shard_map(_body, mesh=mesh, in_specs=specs,
                           out_specs=out_specs, check_rep=False),
                 keep_unused=True)
    # fetch only the smallest output leaf to confirm completion — pulling a
    # big leaf would re-measure the tunnel, not the device
    leaves_sized = lambda out: min(
        jax.tree.leaves(out), key=lambda x: np.prod(x.shape))
    out = fn(*concat_in, *concat_zero)
    jax.block_until_ready(out)
    _ = np.asarray(leaves_sized(out)).ravel()[0]
    times = []
    for _ in range(reps):
        t0 = time.perf_counter()
        out = fn(*concat_in, *concat_zero)
        jax.block_until_ready(out)
        _ = np.asarray(leaves_sized(out)).ravel()[0]
        times.append(time.perf_counter() - t0)
    return min(times), sorted(times)[:4]


if __name__ == "__main__":
    import sys
    sys.path.insert(0, "/root/problem")
    import reference
    inputs = {k: np.asarray(v) for k, v in reference.setup_inputs().items()}
    idx, vals = kernel(**inputs)
    print("kernel ran:", idx.shape, vals.shape)
